# revision 21
# baseline (speedup 1.0000x reference)
"""Trainium2 Bass kernel for nn_DiagSSMBlock.

Math: s = x @ B  (T=4096, H=2048); h_t = a * h_{t-1} + s_t per channel
(equivalent to the reference depthwise causal conv with kernel a^t, since
|a| <= sqrt(2/H) ~= 0.031 the kernel decays below fp32 denormals within
~16 taps).  Output: (1, T, H).

Sharding: data-parallel over T across 8 cores; each core computes 512
timesteps (plus W=16 warm-up rows to rebuild the scan carry, exact to
fp32: a^17 ~= 2.6e-26).  Every core streams the full B.

Per-core device pipeline:
  - x chunk is pre-transposed on the host (sharding layout prep) into
    xT[p, k, t] = x[t, 128k + p], so the GEMM contraction dim lands on
    SBUF partitions with no on-device transpose.
  - GEMM: for each of 16 output-channel tiles m, accumulate 16 k-tile
    matmuls into PSUM (fp32r, moving free dim 264 >= 256 -> full PE rate).
  - Scan: tensor_tensor_scan (DVE) state = a*state + s straight out of
    PSUM into SBUF, chained across the two 264-wide chunks.
  - Output stays channel-major (h^T) on device; the host unshard
    restores (T, H) layout while gathering the 8 T-chunks.
"""

from contextlib import ExitStack

import numpy as np

T_FULL, H = 4096, 2048
N_CORES = 8
T_CHUNK = T_FULL // N_CORES  # 512
W = 16  # scan warm-up rows
T_SPAN = T_CHUNK + W  # 528
HALF = T_SPAN // 2  # 264 (>= 256 keeps fp32r matmul at full rate)
KT = H // 128  # 16 contraction tiles
MT = H // 128  # 16 output-channel tiles
# xT piece sizes in k-slabs: two single-slab pieces first so the first
# matmuls start as early as possible, then full pairs.
PIECES = [1, 1, 2, 2, 2, 2, 2, 2, 2]
PSTART = [sum(PIECES[:i]) for i in range(len(PIECES))]
XP = len(PIECES)

_CACHE = {}


def _build():
    import concourse.mybir as mybir
    import concourse.tile as tile
    from concourse import bacc

    f32 = mybir.dt.float32
    f32r = mybir.dt.float32r

    nc = bacc.Bacc("TRN2", target_bir_lowering=False, debug=False, num_devices=N_CORES)
    xT = nc.dram_tensor("xT", [128, KT, T_SPAN], f32r, kind="ExternalInput").ap()
    Bm = nc.dram_tensor("Bm", [MT, 128, KT, 128], f32r, kind="ExternalInput").ap()
    a = nc.dram_tensor("a", [128, MT], f32, kind="ExternalInput").ap()
    out = nc.dram_tensor("out", [MT, 128, T_CHUNK], f32, kind="ExternalOutput").ap()

    with tile.TileContext(nc) as tc, ExitStack() as ctx:
        const = ctx.enter_context(tc.tile_pool(name="const", bufs=1))
        xt_pool = ctx.enter_context(tc.tile_pool(name="xt", bufs=XP))
        b_pool = ctx.enter_context(tc.tile_pool(name="bm", bufs=8))
        ht_pool = ctx.enter_context(tc.tile_pool(name="ht", bufs=3))
        ps_gemm = ctx.enter_context(tc.tile_pool(name="psg", bufs=8, space="PSUM"))

        a_sb = const.tile([128, MT], f32)
        nc.sync.dma_start(out=a_sb, in_=a)

        rings = [nc.sync, nc.scalar]

        bms = {}
        xts = [None] * XP

        def load_xt(q, ring):
            n = PIECES[q]
            t = xt_pool.tile(
                [128, n * T_SPAN], f32r, tag=f"xt{q}", bufs=1, name=f"xt{q}"
            )
            ring.dma_start(
                out=t[:].rearrange("p (k t) -> p k t", k=n),
                in_=xT[:, PSTART[q] : PSTART[q] + n, :],
            )
            xts[q] = t

        def load_bm_part(m, lo, hi, ring):
            if m not in bms:
                bms[m] = b_pool.tile([128, KT * 128], f32r, tag="bm", name=f"bm{m}")
            ring.dma_start(
                out=bms[m][:, lo * 128 : hi * 128].rearrange(
                    "p (k c) -> p k c", k=hi - lo
                ),
                in_=Bm[m, :, lo:hi, :],
            )

        # Ring FIFO plan: first matmul needs bm0[k0:2] + xt piece 0; B
        # slabs for m=1..3 drip in between xT pieces so each m-tile can
        # join the phase-1 interleave shortly after the previous.
        load_bm_part(0, 0, 2, nc.sync)
        load_xt(0, nc.scalar)
        load_xt(1, nc.sync)
        load_bm_part(0, 2, 8, nc.scalar)
        load_bm_part(0, 8, 16, nc.sync)
        load_xt(2, nc.scalar)
        load_xt(3, nc.sync)
        load_bm_part(1, 0, 8, nc.scalar)
        load_bm_part(1, 8, 16, nc.sync)
        load_xt(4, nc.scalar)
        load_xt(5, nc.sync)
        load_bm_part(2, 0, 8, nc.scalar)
        load_bm_part(2, 8, 16, nc.sync)
        load_xt(6, nc.scalar)
        load_xt(7, nc.sync)
        load_bm_part(3, 0, 8, nc.scalar)
        load_bm_part(3, 8, 16, nc.sync)
        load_xt(8, nc.scalar)

        def xt_slice(k, lo, hi):
            q = max(i for i in range(XP) if PSTART[i] <= k)
            r = k - PSTART[q]
            return xts[q][:, r * T_SPAN + lo : r * T_SPAN + hi]

        PH1 = 4  # m-tiles processed k-outer during the input-load ramp

        def emit_mm(ps, m, k, lo, hi):
            nc.tensor.matmul(
                ps[:],
                bms[m][:, k * 128 : (k + 1) * 128],
                xt_slice(k, lo, hi),
                start=(k == 0),
                stop=(k == KT - 1),
            )

        def emit_scan_out(m, psA, psB):
            ht = ht_pool.tile([128, T_SPAN], f32, tag="ht", name=f"ht{m}")
            a_bc = a_sb[:, m : m + 1].broadcast_to([128, HALF])
            nc.vector.tensor_tensor_scan(
                ht[:, 0:HALF], a_bc, psA[:], 0.0,
                mybir.AluOpType.mult, mybir.AluOpType.add,
            )
            rings[m % 2].dma_start(out=out[m, :, 0 : HALF - W], in_=ht[:, W:HALF])
            if m < MT - 1:
                nc.vector.tensor_tensor_scan(
                    ht[:, HALF:T_SPAN], a_bc, psB[:], ht[:, HALF - 1 : HALF],
                    mybir.AluOpType.mult, mybir.AluOpType.add,
                )
                rings[m % 2].dma_start(
                    out=out[m, :, HALF - W : T_CHUNK], in_=ht[:, HALF:T_SPAN]
                )
            else:
                # last m-tile: split the trailing scan+store so the final
                # dependency chain after the last matmul is half as long
                q3 = HALF + HALF // 2
                a_bc_h = a_sb[:, m : m + 1].broadcast_to([128, HALF // 2])
                nc.vector.tensor_tensor_scan(
                    ht[:, HALF:q3], a_bc_h, psB[:, 0 : HALF // 2],
                    ht[:, HALF - 1 : HALF],
                    mybir.AluOpType.mult, mybir.AluOpType.add,
                )
                rings[m % 2].dma_start(
                    out=out[m, :, HALF - W : q3 - W], in_=ht[:, HALF:q3]
                )
                nc.vector.tensor_tensor_scan(
                    ht[:, q3:T_SPAN], a_bc_h, psB[:, HALF // 2 : HALF],
                    ht[:, q3 - 1 : q3],
                    mybir.AluOpType.mult, mybir.AluOpType.add,
                )
                rings[(m + 1) % 2].dma_start(
                    out=out[m, :, q3 - W : T_CHUNK], in_=ht[:, q3:T_SPAN]
                )

        # Phase 1: m-tiles 0..3 accumulate k-outer following the xT piece
        # arrival order; m joins the rotation one piece late per index
        # (its B slab lands that much later) and catches up on the
        # already-resident backlog pieces.
        # HAM warm-up filler matmuls read already-landed bm0 data (f32r);
        # interleaved into the first phase-1 pieces they bridge the
        # DMA-paced gaps so the PE clock gate reaches 2.4 GHz early.
        ps_warm = ps_gemm.tile([128, HALF], f32, tag="ps", name="ps_warm")
        ph1 = {}
        for m in range(PH1):
            ph1[m] = (
                ps_gemm.tile([128, HALF], f32, tag="ps", name=f"psA{m}"),
                ps_gemm.tile([128, HALF], f32, tag="ps", name=f"psB{m}"),
            )
        FILLERS = {0: 5, 1: 2, 2: 1}
        emitted = [0] * PH1  # next k to emit per phase-1 m
        for q in range(XP):
            avail_k = PSTART[q] + PIECES[q]
            for m in range(min(q + 1, PH1)):
                for k in range(emitted[m], avail_k):
                    emit_mm(ph1[m][0], m, k, 0, HALF)
                    emit_mm(ph1[m][1], m, k, HALF, T_SPAN)
                emitted[m] = avail_k
            for _ in range(FILLERS.get(q, 0)):
                nc.tensor.matmul(
                    ps_warm[:, 0:256],
                    bms[0][:, 0:128],
                    bms[0][:, 0:256],
                    start=True,
                    stop=True,
                )
        # prefetch the first phase-2 B slabs while phase-1 finishes
        load_bm_part(PH1, 0, 8, nc.sync)
        load_bm_part(PH1, 8, 16, nc.scalar)
        load_bm_part(PH1 + 1, 0, 8, nc.sync)
        load_bm_part(PH1 + 1, 8, 16, nc.scalar)
        for m in range(PH1):
            emit_scan_out(m, *ph1[m])

        # Phase 2: remaining m-tiles run dense, k-inner; B slabs stream
        # two m ahead, alternating rings.
        for m in range(PH1, MT):
            if m + 2 < MT:
                load_bm_part(m + 2, 0, 8, rings[m % 2])
                load_bm_part(m + 2, 8, 16, rings[(m + 1) % 2])
            psA = ps_gemm.tile([128, HALF], f32, tag="ps", name=f"psA{m}")
            psB = ps_gemm.tile([128, HALF], f32, tag="ps", name=f"psB{m}")
            for k in range(KT):
                emit_mm(psA, m, k, 0, HALF)
            for k in range(KT):
                emit_mm(psB, m, k, HALF, T_SPAN)
            emit_scan_out(m, psA, psB)

    nc.compile()
    return nc


def _get_nc():
    if "nc" not in _CACHE:
        _CACHE["nc"] = _build()
    return _CACHE["nc"]


def _shard_inputs(x, a, B):
    x = np.ascontiguousarray(x, dtype=np.float32)
    a = np.ascontiguousarray(a, dtype=np.float32)
    B = np.ascontiguousarray(B, dtype=np.float32)
    B_lin = np.ascontiguousarray(
        B.reshape(KT, 128, MT, 128).transpose(2, 1, 0, 3)
    )  # [m, p, k, c] = B[128k+p, 128m+c]
    a_lin = np.ascontiguousarray(a.reshape(MT, 128).T)  # [p, m] = a[128m+p]
    xp = np.concatenate([np.zeros((W, H), np.float32), x], axis=0)
    in_maps = []
    for c in range(N_CORES):
        chunk = xp[c * T_CHUNK : c * T_CHUNK + T_SPAN]  # (T_SPAN, H)
        xT_lin = np.ascontiguousarray(
            chunk.T.reshape(KT, 128, T_SPAN).transpose(1, 0, 2)
        )  # [p, k, t] = x[t, 128k+p]
        in_maps.append({"xT": xT_lin, "Bm": B_lin, "a": a_lin})
    return in_maps


def _gather_output(results):
    out = np.empty((T_FULL, H), np.float32)
    for c in range(N_CORES):
        o = results[c]["out"]  # (MT, 128, T_CHUNK): h^T[chan, t_local]
        out[c * T_CHUNK : (c + 1) * T_CHUNK] = o.reshape(H, T_CHUNK).T
    return out[None]


def _run(inputs, trace=False):
    from concourse import bass_utils

    nc = _get_nc()
    in_maps = _shard_inputs(inputs["x"], inputs["a"], inputs["B"])
    res = bass_utils.run_bass_kernel_spmd(
        nc, in_maps, core_ids=list(range(N_CORES)), trace=trace
    )
    return _gather_output(res.results), res


def kernel(x, a, B):
    out, _ = _run({"x": x, "a": a, "B": B})
    return out
